# revision 17
# baseline (speedup 1.0000x reference)
"""BiDAF-style attention kernel for Trainium2, 8-core data-parallel over batch.

Problem (per batch b):
  sim[c,q] = ctx[c]@w_c + qry[q]@w_q + sum_h ctx[c,h] w_m[h] qry[q,h] + att_b
  alpha = softmax_q(sim);        a[c] = sum_q alpha[c,q] qry[q]
  beta  = softmax_c(max_q sim);  bv   = sum_c beta[c] ctx[c]
  out = [ctx | a | ctx*a | ctx*bv]          (C, 4H)

Memory-bound problem: 5.125 MB of HBM traffic per batch per core (1.125 in,
4 out) -> ~14.3 us/batch at the 358 GB/s per-core HBM limit. All compute is
sized to hide under that.

Key algebra (same as v1):
  - att_b is a global constant -> cancels everywhere; dropped.
  - No max subtraction inside softmax: logits are O(4), exp is safe.
  - cvec = ctx@w_c is accumulated into simT via a ones-broadcast matmul so
    exp(cvec) is embedded in es and the beta path needs no extra layout work.
  - max_q exp(sim) = exp(max_q sim): beta max taken on the exp'd values.

v2 changes (all aimed at engine busy-time, the DMA floor was already near):
  - All transposes are REGULAR matmuls against an identity rhs (not
    transpose-mode): ~110ns warm vs ~275ns, and they count as PE activity so
    the HAM clock-gate stays at 8/8 (transpose-mode does not).
  - es and the a-matmul are f16 (was f32r): FWL weight loads + single pass.
  - out[:, 0:H] = ctx is a DRAM->DRAM DMA (no SBUF bounce, 8 MB saved).
  - att_w loaded as [3,256]/[6,128] contiguous HWDGE descriptors and
    rearranged on-chip (was: degenerate SWDGE patterns costing ~10us).
  - ctx f32->f16 cast is one ACT op; qvec is one fused mul+reduce DVE op;
    static parts of qaug/ctx_h written once per pool buffer.
  - a|ctx*a written as one [128, ct, 2, H] slab, DMA'd in 1 MB halves.
"""

import numpy as np

import concourse.bass as bass
import concourse.tile as tile
from concourse import mybir
from concourse.alu_op_type import AluOpType
from concourse.bass_utils import run_bass_kernel_spmd
from concourse.masks import make_identity

B, C, Q, H = 64, 1024, 128, 256
NCORES = 8
BL = B // NCORES          # batches per core
CT = C // 128             # context row-tiles per batch
F32 = mybir.dt.float32
F16 = mybir.dt.float16


def split_waits(nc, max_waits=1):
    """walrus codegen in this container rejects >1 sem wait per instruction;
    move excess waits onto same-engine NoOps inserted just before."""
    n_new = 0
    for f in nc.m.functions:
        for blk in f.blocks:
            out = []
            for ins in blk.instructions:
                waits = list(ins.sync_info.on_wait) if ins.sync_info else []
                if len(waits) > max_waits:
                    extra, keep = waits[:-max_waits], waits[-max_waits:]
                    for j in range(0, len(extra), max_waits):
                        nop = mybir.InstNoOp(name=f"I-wsplit-{n_new}", ins=[], outs=[])
                        n_new += 1
                        nop.engine = ins.engine
                        nop.sync_info = mybir.SyncInfo(
                            on_wait=list(extra[j : j + max_waits]), on_update=[]
                        )
                        out.append(nop)
                    ins.sync_info.on_wait = list(keep)
                out.append(ins)
            blk.instructions = out
    return n_new


def build():
    nc = bass.Bass()
    ctx_d = nc.dram_tensor("context", [BL, C, H], F32, kind="ExternalInput")
    q_d = nc.dram_tensor("query", [BL, Q, H], F32, kind="ExternalInput")
    w_d = nc.dram_tensor("att_w", [3 * H], F32, kind="ExternalInput")
    b_d = nc.dram_tensor("att_b", [1], F32, kind="ExternalInput")
    out_d = nc.dram_tensor("out", [BL, C, 4 * H], F32, kind="ExternalOutput")

    X = mybir.AxisListType.X
    EXP = mybir.ActivationFunctionType.Exp
    NAF = H + 2 + 128     # a-matmul psum width: [a | S | S | esT]

    with tile.TileContext(nc) as tc:
        from contextlib import ExitStack

        with ExitStack() as ctx:
            consts = ctx.enter_context(tc.tile_pool(name="consts", bufs=1))
            ctxhp = ctx.enter_context(tc.tile_pool(name="ctxh", bufs=2))
            ctxTp = ctx.enter_context(tc.tile_pool(name="ctxT", bufs=2))
            qp = ctx.enter_context(tc.tile_pool(name="qp", bufs=2))
            qaugp = ctx.enter_context(tc.tile_pool(name="qaug", bufs=2))
            esp = ctx.enter_context(tc.tile_pool(name="es", bufs=2))
            slabp = ctx.enter_context(tc.tile_pool(name="slab", bufs=3))
            cbvp = ctx.enter_context(tc.tile_pool(name="cbv", bufs=2))
            smallp = ctx.enter_context(tc.tile_pool(name="small", bufs=8))
            ps_tp = ctx.enter_context(tc.tile_pool(name="ps_tp", bufs=2, space="PSUM"))
            ps_sim = ctx.enter_context(tc.tile_pool(name="ps_sim", bufs=1, space="PSUM"))
            ps_a = ctx.enter_context(tc.tile_pool(name="ps_a", bufs=3, space="PSUM"))
            ps_sm = ctx.enter_context(tc.tile_pool(name="ps_sm", bufs=1, space="PSUM"))

            ident = consts.tile([128, 128], F32)
            make_identity(nc, ident[:, :])
            ident_h = consts.tile([128, 128], F16)
            nc.vector.tensor_copy(ident_h[:, :], ident[:, :])
            ones_col = consts.tile([128, 1], F32)
            nc.vector.memset(ones_col[:, :], 1.0)
            ones_row = consts.tile([1, 128], F32)
            nc.vector.memset(ones_row[:, :], 1.0)
            ones_row_h = consts.tile([1, 128], F16)
            nc.vector.memset(ones_row_h[:, :], 1.0)

            # att_w via contiguous HWDGE loads:
            # [6,128] rows (w_c h0|h1, w_q h0|h1, w_m h0|h1), [1,256] w_q
            w6 = consts.tile([6, 128], F32)
            nc.sync.dma_start(
                out=w6[:, :],
                in_=bass.AP(tensor=w_d, offset=0, ap=[[128, 6], [1, 128]]),
            )
            # wcols [128, 6] via PE transpose of w6
            wtp = ps_tp.tile([128, 6], F32, tag="tp")
            nc.tensor.matmul(
                wtp[:, :], lhsT=w6[:, :], rhs=ident[0:6, 0:6],
                start=True, stop=True, is_transpose=True,
            )
            wm_col = consts.tile([128, 2], F32)      # w_m halves, f32 col
            nc.vector.tensor_copy(wm_col[:, :], wtp[:, 4:6])
            wc_col_h = consts.tile([128, 2], F16)    # w_c halves, f16 col
            nc.vector.tensor_copy(wc_col_h[:, :], wtp[:, 0:2])
            # wqb [128, H] f32: broadcast of w_q for the qvec row-reduction
            wq_row = consts.tile([1, H], F32)
            nc.sync.dma_start(
                out=wq_row[:, :],
                in_=bass.AP(tensor=w_d, offset=H, ap=[[H, 1], [1, H]]),
            )
            wqp = ps_tp.tile([128, H], F32, tag="tp")
            nc.tensor.matmul(
                wqp[:, :], lhsT=ones_row[:, :], rhs=wq_row[:, :],
                start=True, stop=True,
            )
            wqb = consts.tile([128, H], F32)
            nc.vector.tensor_copy(wqb[:, :], wqp[:, :])

            for b in range(BL):
                # ---- loads (scalar ring). ctx lands directly in the output
                # slab [ctx | a | ctx*a] so the store is one 3KB-contiguous
                # row per (p, ct) and the ctx passthrough rides along. ----
                slab = slabp.tile([128, CT, 3 * H], F32, tag="slab")
                ctx_sb = slab[:, :, 0:H]
                nc.scalar.dma_start(
                    out=ctx_sb,
                    in_=ctx_d[b].rearrange("(ct p) h -> p ct h", p=128),
                )
                q_sb = qp.tile([128, H], F32, tag="q")
                nc.scalar.dma_start(out=q_sb[:, :], in_=q_d[b])

                # ---- qaug = [q_h | 1 | 1 | ident] (f16) ----
                qaug = qaugp.tile([128, NAF], F16, tag="qaug")
                if b < 2:
                    ones2 = bass.AP(
                        tensor=ones_col.tensor,
                        offset=ones_col[:, :].offset,
                        ap=[ones_col[:, :].ap[0], [0, 2]],
                    )
                    nc.vector.tensor_copy(qaug[:, H : H + 2], ones2)
                    nc.vector.tensor_copy(qaug[:, H + 2 : NAF], ident_h[:, :])
                nc.scalar.copy(qaug[:, 0:H], q_sb[:, :])

                # ---- qT scaled by w_m (transpose via matmul w/ identity) ----
                qTs_h = qp.tile([128, 2, 128], F16, tag="qts")
                for ht in range(2):
                    tq = ps_tp.tile([128, 128], F32, tag="tp")
                    nc.tensor.matmul(
                        tq[:, :], lhsT=qaug[:, ht * 128 : (ht + 1) * 128],
                        rhs=ident_h[:, :], start=True, stop=True,
                    )
                    nc.vector.tensor_scalar_mul(
                        qTs_h[:, ht, :], tq[:, :], wm_col[:, ht : ht + 1]
                    )

                # ---- qvec[q] = qry[q] @ w_q (fused mul+reduce) ----
                scr = qp.tile([128, H], F32, tag="scr")
                qvec = smallp.tile([128, 1], F32, tag="qvec")
                nc.vector.tensor_mul(scr[:, :], q_sb[:, :], wqb[:, :])
                nc.vector.reduce_sum(qvec[:, :], scr[:, :], axis=X)

                # ---- ctx f16 (+ ones cols for the bv normalizer), cast in
                # halves interleaved with the transposes so the PE starts
                # after the first half ----
                ctx_h = ctxhp.tile([128, CT, H + 2], F16, tag="ctxh")
                if b < 2:
                    ones_b = bass.AP(
                        tensor=ones_col.tensor,
                        offset=ones_col[:, :].offset,
                        ap=[ones_col[:, :].ap[0], [0, CT], [0, 2]],
                    )
                    nc.vector.tensor_copy(ctx_h[:, :, H : H + 2], ones_b)

                # ---- ctxT (16 tile transposes via matmul w/ identity) ----
                ctxT_h = ctxTp.tile([128, 2, C], F16, tag="ctxT")
                for half in range(2):
                    cts = range(half * CT // 2, (half + 1) * CT // 2)
                    nc.scalar.copy(
                        ctx_h[:, cts.start : cts.stop, 0:H],
                        slab[:, cts.start : cts.stop, 0:H],
                    )
                    for ct in cts:
                        # both ht transposes of one ct share a PSUM bank ->
                        # one [128, 2, 128] copy out per ct
                        tp = ps_tp.tile([128, 256], F32, tag="tp")
                        for ht in range(2):
                            nc.tensor.matmul(
                                tp[:, ht * 128 : (ht + 1) * 128],
                                lhsT=ctx_h[:, ct, ht * 128 : (ht + 1) * 128],
                                rhs=ident_h[:, :], start=True, stop=True,
                                skip_group_check=(ht == 1),
                            )
                        dst = bass.AP(
                            tensor=ctxT_h.tensor,
                            offset=ctxT_h[:, 0, ct * 128 : (ct + 1) * 128].offset,
                            ap=[ctxT_h[:, 0, ct * 128 :].ap[0], [C, 2], [1, 128]],
                        )
                        if ct % 2 == 0:
                            nc.scalar.copy(dst, tp[:, :])
                        else:
                            nc.vector.tensor_copy(dst, tp[:, :])

                # ---- cvec row: w_c^T @ ctxT ----
                cvec_h = smallp.tile([1, C], F16, tag="cvec")
                for ch in range(2):
                    cvr = ps_tp.tile([1, 512], F32, tag="tp")
                    for ht in range(2):
                        nc.tensor.matmul(
                            cvr[:, :], lhsT=wc_col_h[:, ht : ht + 1],
                            rhs=ctxT_h[:, ht, ch * 512 : (ch + 1) * 512],
                            start=(ht == 0), stop=(ht == 1),
                        )
                    nc.scalar.copy(cvec_h[:, ch * 512 : (ch + 1) * 512], cvr[:, :])

                # ---- simT = qTs^T @ ctxT + 1 (x) cvec; es = exp(simT+qvec) ----
                es_h = esp.tile([128, C], F16, tag="es")
                for ch in range(2):
                    simp = ps_sim.tile([128, 512], F32, tag=f"sim{ch}")
                    for ht in range(2):
                        nc.tensor.matmul(
                            simp[:, :], lhsT=qTs_h[:, ht, :],
                            rhs=ctxT_h[:, ht, ch * 512 : (ch + 1) * 512],
                            start=(ht == 0), stop=False,
                        )
                    nc.tensor.matmul(
                        simp[:, :], lhsT=ones_row_h[:, :],
                        rhs=cvec_h[:, ch * 512 : (ch + 1) * 512],
                        start=False, stop=True,
                    )
                    nc.scalar.activation(
                        out=es_h[:, ch * 512 : (ch + 1) * 512], in_=simp[:, :],
                        func=EXP, bias=qvec[:, 0:1], scale=1.0,
                    )

                # ---- per-ct: a | ctx*a | beta max | bv accumulation ----
                M8w = smallp.tile([128, CT], F16, tag="m8")
                bv_ps = ps_sm.tile([1, H + 2], F32, tag="bv")
                for ct in range(CT):
                    af = ps_a.tile([128, NAF], F32, tag="a")
                    nc.tensor.matmul(
                        af[:, :], lhsT=es_h[:, ct * 128 : (ct + 1) * 128],
                        rhs=qaug[:, :], start=True, stop=True,
                    )
                    rS = smallp.tile([128, 1], F32)
                    nc.vector.reciprocal(rS[:, :], af[:, H : H + 1])
                    nc.vector.tensor_scalar_mul(
                        slab[:, ct, H : 2 * H], af[:, 0:H], rS[:, :]
                    )
                    nc.gpsimd.tensor_mul(
                        slab[:, ct, 2 * H : 3 * H], slab[:, ct, 0:H],
                        slab[:, ct, H : 2 * H],
                    )
                    nc.vector.reduce_max(
                        M8w[:, ct : ct + 1], af[:, H + 2 : NAF], axis=X
                    )
                    nc.tensor.matmul(
                        bv_ps[:, :], lhsT=M8w[:, ct : ct + 1],
                        rhs=ctx_h[:, ct, :],
                        start=(ct == 0), stop=(ct == CT - 1),
                        skip_group_check=True,
                    )
                    if ct == 3:
                        nc.sync.dma_start(
                            out=out_d[b, 0:512, 0 : 3 * H].rearrange(
                                "(ct p) h -> p ct h", p=128
                            ),
                            in_=slab[:, 0:4, :],
                        )
                nc.sync.dma_start(
                    out=out_d[b, 512:1024, 0 : 3 * H].rearrange(
                        "(ct p) h -> p ct h", p=128
                    ),
                    in_=slab[:, 4:8, :],
                )

                # ---- beta tail: bv normalize, broadcast, ctx*bv ----
                rSb = smallp.tile([1, 1], F32)
                nc.vector.reciprocal(rSb[:, :], bv_ps[:, H : H + 1])
                bv_h = smallp.tile([1, H], F16, tag="bvh")
                nc.vector.tensor_scalar_mul(bv_h[:, :], bv_ps[:, 0:H], rSb[:, :])
                bb_ps = ps_a.tile([128, NAF], F32, tag="a")
                nc.tensor.matmul(
                    bb_ps[:, 0:H], lhsT=ones_row_h[:, :], rhs=bv_h[:, :],
                    start=True, stop=True,
                )
                bb_bcast = bass.AP(
                    tensor=bb_ps.tensor,
                    offset=bb_ps[:, 0:H].offset,
                    ap=[bb_ps[:, 0:H].ap[0], [0, CT], [1, H]],
                )
                cbv8 = cbvp.tile([128, CT, H], F32, tag="cbv")
                nc.vector.tensor_mul(cbv8[:, :, :], slab[:, :, 0:H], bb_bcast)
                nc.gpsimd.dma_start(
                    out=out_d[b, :, 3 * H : 4 * H].rearrange(
                        "(ct p) h -> p ct h", p=128
                    ),
                    in_=cbv8[:, :, :],
                )

    split_waits(nc)
    return nc


_NC = None
LAST_RESULT = None


def kernel(_trace=False, **inputs):
    global _NC, LAST_RESULT
    if _NC is None:
        _NC = build()
    context = np.ascontiguousarray(np.asarray(inputs["context"], dtype=np.float32))
    query = np.ascontiguousarray(np.asarray(inputs["query"], dtype=np.float32))
    att_w = np.ascontiguousarray(np.asarray(inputs["att_w"], dtype=np.float32))
    att_b = np.asarray(inputs["att_b"], dtype=np.float32).reshape(1)
    in_maps = [
        {
            "context": np.ascontiguousarray(context[i * BL : (i + 1) * BL]),
            "query": np.ascontiguousarray(query[i * BL : (i + 1) * BL]),
            "att_w": att_w,
            "att_b": att_b,
        }
        for i in range(NCORES)
    ]
    res = run_bass_kernel_spmd(
        _NC, in_maps, core_ids=list(range(NCORES)), trace=_trace
    )
    LAST_RESULT = res
    return np.concatenate([r["out"] for r in res.results], axis=0)


# revision 19
# speedup vs baseline: 1.2170x; 1.2170x over previous
"""BiDAF-style attention kernel for Trainium2, 8-core data-parallel over batch.

Problem (per batch b):
  sim[c,q] = ctx[c]@w_c + qry[q]@w_q + sum_h ctx[c,h] w_m[h] qry[q,h] + att_b
  alpha = softmax_q(sim);        a[c] = sum_q alpha[c,q] qry[q]
  beta  = softmax_c(max_q sim);  bv   = sum_c beta[c] ctx[c]
  out = [ctx | a | ctx*a | ctx*bv]          (C, 4H)

Memory-bound problem: 5.125 MB of HBM traffic per batch per core (1.125 in,
4 out) -> ~14.3 us/batch at the 358 GB/s per-core HBM limit. All compute is
sized to hide under that.

Key algebra (same as v1):
  - att_b is a global constant -> cancels everywhere; dropped.
  - No max subtraction inside softmax: logits are O(4), exp is safe.
  - cvec = ctx@w_c is accumulated into simT via a ones-broadcast matmul so
    exp(cvec) is embedded in es and the beta path needs no extra layout work.
  - max_q exp(sim) = exp(max_q sim): beta max taken on the exp'd values.

v2 changes (all aimed at engine busy-time, the DMA floor was already near):
  - All transposes are REGULAR matmuls against an identity rhs (not
    transpose-mode): ~110ns warm vs ~275ns, and they count as PE activity so
    the HAM clock-gate stays at 8/8 (transpose-mode does not).
  - es and the a-matmul are f16 (was f32r): FWL weight loads + single pass.
  - out[:, 0:H] = ctx is a DRAM->DRAM DMA (no SBUF bounce, 8 MB saved).
  - att_w loaded as [3,256]/[6,128] contiguous HWDGE descriptors and
    rearranged on-chip (was: degenerate SWDGE patterns costing ~10us).
  - ctx f32->f16 cast is one ACT op; qvec is one fused mul+reduce DVE op;
    static parts of qaug/ctx_h written once per pool buffer.
  - a|ctx*a written as one [128, ct, 2, H] slab, DMA'd in 1 MB halves.
"""

import numpy as np

import concourse.bass as bass
import concourse.tile as tile
from concourse import mybir
from concourse.alu_op_type import AluOpType
from concourse.bass_utils import run_bass_kernel_spmd
from concourse.masks import make_identity

B, C, Q, H = 64, 1024, 128, 256
NCORES = 8
BL = B // NCORES          # batches per core
CT = C // 128             # context row-tiles per batch
F32 = mybir.dt.float32
F16 = mybir.dt.float16


def split_waits(nc, max_waits=1):
    """walrus codegen in this container rejects >1 sem wait per instruction;
    move excess waits onto same-engine NoOps inserted just before."""
    n_new = 0
    for f in nc.m.functions:
        for blk in f.blocks:
            out = []
            for ins in blk.instructions:
                waits = list(ins.sync_info.on_wait) if ins.sync_info else []
                if len(waits) > max_waits:
                    extra, keep = waits[:-max_waits], waits[-max_waits:]
                    for j in range(0, len(extra), max_waits):
                        nop = mybir.InstNoOp(name=f"I-wsplit-{n_new}", ins=[], outs=[])
                        n_new += 1
                        nop.engine = ins.engine
                        nop.sync_info = mybir.SyncInfo(
                            on_wait=list(extra[j : j + max_waits]), on_update=[]
                        )
                        out.append(nop)
                    ins.sync_info.on_wait = list(keep)
                out.append(ins)
            blk.instructions = out
    return n_new


def build():
    nc = bass.Bass()
    ctx_d = nc.dram_tensor("context", [BL, C, H], F32, kind="ExternalInput")
    q_d = nc.dram_tensor("query", [BL, Q, H], F32, kind="ExternalInput")
    w_d = nc.dram_tensor("att_w", [3 * H], F32, kind="ExternalInput")
    b_d = nc.dram_tensor("att_b", [1], F32, kind="ExternalInput")
    out_d = nc.dram_tensor("out", [BL, C, 4 * H], F32, kind="ExternalOutput")

    X = mybir.AxisListType.X
    EXP = mybir.ActivationFunctionType.Exp
    NAF = H + 2 + 128     # a-matmul psum width: [a | S | S | esT]

    with tile.TileContext(nc) as tc:
        from contextlib import ExitStack

        with ExitStack() as ctx:
            consts = ctx.enter_context(tc.tile_pool(name="consts", bufs=1))
            ctxhp = ctx.enter_context(tc.tile_pool(name="ctxh", bufs=2))
            ctxTp = ctx.enter_context(tc.tile_pool(name="ctxT", bufs=2))
            qp = ctx.enter_context(tc.tile_pool(name="qp", bufs=2))
            qaugp = ctx.enter_context(tc.tile_pool(name="qaug", bufs=2))
            esp = ctx.enter_context(tc.tile_pool(name="es", bufs=2))
            slabp = ctx.enter_context(tc.tile_pool(name="slab", bufs=4))
            cbvp = ctx.enter_context(tc.tile_pool(name="cbv", bufs=3))
            smallp = ctx.enter_context(tc.tile_pool(name="small", bufs=8))
            ps_tp = ctx.enter_context(tc.tile_pool(name="ps_tp", bufs=2, space="PSUM"))
            ps_sim = ctx.enter_context(tc.tile_pool(name="ps_sim", bufs=1, space="PSUM"))
            ps_a = ctx.enter_context(tc.tile_pool(name="ps_a", bufs=3, space="PSUM"))
            ps_sm = ctx.enter_context(tc.tile_pool(name="ps_sm", bufs=1, space="PSUM"))

            ident = consts.tile([128, 128], F32)
            make_identity(nc, ident[:, :])
            ident_h = consts.tile([128, 128], F16)
            nc.vector.tensor_copy(ident_h[:, :], ident[:, :])
            ones_col = consts.tile([128, 1], F32)
            nc.vector.memset(ones_col[:, :], 1.0)
            ones_row = consts.tile([1, 128], F32)
            nc.vector.memset(ones_row[:, :], 1.0)
            ones_row_h = consts.tile([1, 128], F16)
            nc.vector.memset(ones_row_h[:, :], 1.0)

            # att_w via contiguous HWDGE loads:
            # [6,128] rows (w_c h0|h1, w_q h0|h1, w_m h0|h1), [1,256] w_q
            w6 = consts.tile([6, 128], F32)
            nc.sync.dma_start(
                out=w6[:, :],
                in_=bass.AP(tensor=w_d, offset=0, ap=[[128, 6], [1, 128]]),
            )
            # wcols [128, 6] via PE transpose of w6
            wtp = ps_tp.tile([128, 6], F32, tag="tp")
            nc.tensor.matmul(
                wtp[:, :], lhsT=w6[:, :], rhs=ident[0:6, 0:6],
                start=True, stop=True, is_transpose=True,
            )
            wm_col = consts.tile([128, 2], F32)      # w_m halves, f32 col
            nc.vector.tensor_copy(wm_col[:, :], wtp[:, 4:6])
            wc_col_h = consts.tile([128, 2], F16)    # w_c halves, f16 col
            nc.vector.tensor_copy(wc_col_h[:, :], wtp[:, 0:2])
            # wqb [128, H] f32: broadcast of w_q for the qvec row-reduction
            wq_row = consts.tile([1, H], F32)
            nc.sync.dma_start(
                out=wq_row[:, :],
                in_=bass.AP(tensor=w_d, offset=H, ap=[[H, 1], [1, H]]),
            )
            wqp = ps_tp.tile([128, H], F32, tag="tp")
            nc.tensor.matmul(
                wqp[:, :], lhsT=ones_row[:, :], rhs=wq_row[:, :],
                start=True, stop=True,
            )
            wqb = consts.tile([128, H], F32)
            nc.vector.tensor_copy(wqb[:, :], wqp[:, :])

            for b in range(BL):
                # ---- loads (scalar ring). ctx lands directly in the output
                # slab [ctx | a | ctx*a] so the store is one 3KB-contiguous
                # row per (p, ct) and the ctx passthrough rides along. ----
                slab = slabp.tile([128, CT, 3 * H], F32, tag="slab")
                for half in range(2):
                    lo, hi = half * CT // 2, (half + 1) * CT // 2
                    nc.scalar.dma_start(
                        out=slab[:, lo:hi, 0:H],
                        in_=ctx_d[b, lo * 128 : hi * 128].rearrange(
                            "(ct p) h -> p ct h", p=128
                        ),
                    )
                q_sb = qp.tile([128, H], F32, tag="q")
                nc.scalar.dma_start(out=q_sb[:, :], in_=q_d[b])

                # ---- qaug = [q_h | 1 | 1 | ident] (f16) ----
                qaug = qaugp.tile([128, NAF], F16, tag="qaug")
                if b < 2:
                    ones2 = bass.AP(
                        tensor=ones_col.tensor,
                        offset=ones_col[:, :].offset,
                        ap=[ones_col[:, :].ap[0], [0, 2]],
                    )
                    nc.vector.tensor_copy(qaug[:, H : H + 2], ones2)
                    nc.vector.tensor_copy(qaug[:, H + 2 : NAF], ident_h[:, :])
                nc.scalar.copy(qaug[:, 0:H], q_sb[:, :])

                # ---- qT scaled by w_m (transpose via matmul w/ identity) ----
                qTs_h = qp.tile([128, 2, 128], F16, tag="qts")
                for ht in range(2):
                    tq = ps_tp.tile([128, 128], F32, tag="tp")
                    nc.tensor.matmul(
                        tq[:, :], lhsT=qaug[:, ht * 128 : (ht + 1) * 128],
                        rhs=ident_h[:, :], start=True, stop=True,
                    )
                    nc.vector.tensor_scalar_mul(
                        qTs_h[:, ht, :], tq[:, :], wm_col[:, ht : ht + 1]
                    )

                # ---- qvec[q] = qry[q] @ w_q (fused mul+reduce) ----
                scr = qp.tile([128, H], F32, tag="scr")
                qvec = smallp.tile([128, 1], F32, tag="qvec")
                nc.vector.tensor_mul(scr[:, :], q_sb[:, :], wqb[:, :])
                nc.vector.reduce_sum(qvec[:, :], scr[:, :], axis=X)

                # ---- ctx f16 (+ ones cols for the bv normalizer), cast in
                # halves interleaved with the transposes so the PE starts
                # after the first half ----
                ctx_h = ctxhp.tile([128, CT, H + 2], F16, tag="ctxh")
                if b < 2:
                    ones_b = bass.AP(
                        tensor=ones_col.tensor,
                        offset=ones_col[:, :].offset,
                        ap=[ones_col[:, :].ap[0], [0, CT], [0, 2]],
                    )
                    nc.vector.tensor_copy(ctx_h[:, :, H : H + 2], ones_b)

                # ---- ctxT (16 tile transposes via matmul w/ identity) ----
                ctxT_h = ctxTp.tile([128, 2, C], F16, tag="ctxT")
                for half in range(2):
                    cts = range(half * CT // 2, (half + 1) * CT // 2)
                    nc.scalar.copy(
                        ctx_h[:, cts.start : cts.stop, 0:H],
                        slab[:, cts.start : cts.stop, 0:H],
                    )
                    for ct in cts:
                        # both ht transposes of one ct share a PSUM bank ->
                        # one [128, 2, 128] copy out per ct
                        tp = ps_tp.tile([128, 256], F32, tag="tp")
                        for ht in range(2):
                            nc.tensor.matmul(
                                tp[:, ht * 128 : (ht + 1) * 128],
                                lhsT=ctx_h[:, ct, ht * 128 : (ht + 1) * 128],
                                rhs=ident_h[:, :], start=True, stop=True,
                                skip_group_check=(ht == 1),
                            )
                        dst = bass.AP(
                            tensor=ctxT_h.tensor,
                            offset=ctxT_h[:, 0, ct * 128 : (ct + 1) * 128].offset,
                            ap=[ctxT_h[:, 0, ct * 128 :].ap[0], [C, 2], [1, 128]],
                        )
                        if ct % 2 == 0:
                            nc.scalar.copy(dst, tp[:, :])
                        else:
                            nc.vector.tensor_copy(dst, tp[:, :])

                # ---- cvec row: w_c^T @ ctxT ----
                cvec_h = smallp.tile([1, C], F16, tag="cvec")
                for ch in range(2):
                    cvr = ps_tp.tile([1, 512], F32, tag="tp")
                    for ht in range(2):
                        nc.tensor.matmul(
                            cvr[:, :], lhsT=wc_col_h[:, ht : ht + 1],
                            rhs=ctxT_h[:, ht, ch * 512 : (ch + 1) * 512],
                            start=(ht == 0), stop=(ht == 1),
                        )
                    nc.scalar.copy(cvec_h[:, ch * 512 : (ch + 1) * 512], cvr[:, :])

                # ---- simT = qTs^T @ ctxT + 1 (x) cvec; es = exp(simT+qvec) ----
                es_h = esp.tile([128, C], F16, tag="es")
                for ch in range(2):
                    simp = ps_sim.tile([128, 512], F32, tag=f"sim{ch}")
                    for ht in range(2):
                        nc.tensor.matmul(
                            simp[:, :], lhsT=qTs_h[:, ht, :],
                            rhs=ctxT_h[:, ht, ch * 512 : (ch + 1) * 512],
                            start=(ht == 0), stop=False,
                        )
                    nc.tensor.matmul(
                        simp[:, :], lhsT=ones_row_h[:, :],
                        rhs=cvec_h[:, ch * 512 : (ch + 1) * 512],
                        start=False, stop=True,
                    )
                    nc.scalar.activation(
                        out=es_h[:, ch * 512 : (ch + 1) * 512], in_=simp[:, :],
                        func=EXP, bias=qvec[:, 0:1], scale=1.0,
                    )

                # ---- per-ct: a | ctx*a | beta max | bv accumulation ----
                M8w = smallp.tile([128, CT], F16, tag="m8")
                bv_ps = ps_sm.tile([1, H + 2], F32, tag="bv")
                for ct in range(CT):
                    af = ps_a.tile([128, NAF], F32, tag="a")
                    nc.tensor.matmul(
                        af[:, :], lhsT=es_h[:, ct * 128 : (ct + 1) * 128],
                        rhs=qaug[:, :], start=True, stop=True,
                    )
                    rS = smallp.tile([128, 1], F32)
                    nc.vector.reciprocal(rS[:, :], af[:, H : H + 1])
                    nc.vector.tensor_scalar_mul(
                        slab[:, ct, H : 2 * H], af[:, 0:H], rS[:, :]
                    )
                    nc.gpsimd.tensor_mul(
                        slab[:, ct, 2 * H : 3 * H], slab[:, ct, 0:H],
                        slab[:, ct, H : 2 * H],
                    )
                    nc.vector.reduce_max(
                        M8w[:, ct : ct + 1], af[:, H + 2 : NAF], axis=X
                    )
                    nc.tensor.matmul(
                        bv_ps[:, :], lhsT=M8w[:, ct : ct + 1],
                        rhs=ctx_h[:, ct, :],
                        start=(ct == 0), stop=(ct == CT - 1),
                        skip_group_check=True,
                    )
                    if ct == 3:
                        nc.sync.dma_start(
                            out=out_d[b, 0:512, 0 : 3 * H].rearrange(
                                "(ct p) h -> p ct h", p=128
                            ),
                            in_=slab[:, 0:4, :],
                        )
                nc.sync.dma_start(
                    out=out_d[b, 512:1024, 0 : 3 * H].rearrange(
                        "(ct p) h -> p ct h", p=128
                    ),
                    in_=slab[:, 4:8, :],
                )

                # ---- beta tail: bv normalize, broadcast, ctx*bv ----
                rSb = smallp.tile([1, 1], F32)
                nc.vector.reciprocal(rSb[:, :], bv_ps[:, H : H + 1])
                bv_h = smallp.tile([1, H], F16, tag="bvh")
                nc.vector.tensor_scalar_mul(bv_h[:, :], bv_ps[:, 0:H], rSb[:, :])
                bb_ps = ps_a.tile([128, NAF], F32, tag="a")
                nc.tensor.matmul(
                    bb_ps[:, 0:H], lhsT=ones_row_h[:, :], rhs=bv_h[:, :],
                    start=True, stop=True,
                )
                bb_bcast = bass.AP(
                    tensor=bb_ps.tensor,
                    offset=bb_ps[:, 0:H].offset,
                    ap=[bb_ps[:, 0:H].ap[0], [0, CT], [1, H]],
                )
                cbv8 = cbvp.tile([128, CT, H], F32, tag="cbv")
                nc.vector.tensor_mul(cbv8[:, :, :], slab[:, :, 0:H], bb_bcast)
                nc.gpsimd.dma_start(
                    out=out_d[b, :, 3 * H : 4 * H].rearrange(
                        "(ct p) h -> p ct h", p=128
                    ),
                    in_=cbv8[:, :, :],
                )

    split_waits(nc)
    return nc


_NC = None
LAST_RESULT = None


def kernel(_trace=False, **inputs):
    global _NC, LAST_RESULT
    if _NC is None:
        _NC = build()
    context = np.ascontiguousarray(np.asarray(inputs["context"], dtype=np.float32))
    query = np.ascontiguousarray(np.asarray(inputs["query"], dtype=np.float32))
    att_w = np.ascontiguousarray(np.asarray(inputs["att_w"], dtype=np.float32))
    att_b = np.asarray(inputs["att_b"], dtype=np.float32).reshape(1)
    in_maps = [
        {
            "context": np.ascontiguousarray(context[i * BL : (i + 1) * BL]),
            "query": np.ascontiguousarray(query[i * BL : (i + 1) * BL]),
            "att_w": att_w,
            "att_b": att_b,
        }
        for i in range(NCORES)
    ]
    res = run_bass_kernel_spmd(
        _NC, in_maps, core_ids=list(range(NCORES)), trace=_trace
    )
    LAST_RESULT = res
    return np.concatenate([r["out"] for r in res.results], axis=0)
